# revision 30
# baseline (speedup 1.0000x reference)
"""GCN (3-layer + MLP head) on 8 Trainium2 NeuronCores.

Strategy (graph-parallel, dst-sharded, SWDGE-gather message passing):
  - Host renumbers nodes (LPT bin-packing on in-degree) so every 128-node
    dst block has total in-degree <= 2048; nodes shard 8 ways by new id.
  - Phase A (per core): Hs = dinv * (X_shard @ W1) as bf16 rows, kept in
    SBUF (self-loop term) and AllGathered -> full node table in HBM.
  - MP phase: each dst block's edges are sorted by src table row and split
    into 8 segments of exactly 256 edges (last = remainder); segment k of
    all blocks lies in a fixed ~32k-row window, so int16 gather indices
    cover it with zero per-cell padding.  Chunks of 14 blocks' k-segments
    (3584 idxs) feed dma_gather; per 128-edge group a one-hot S matrix
    (broadcast DVE tensor_tensor is_equal, 4 groups per op) scatter-sums
    messages into the dst block via a PE matmul; each block's 16 groups
    accumulate in PSUM (4 block-slices per bank; start=True only on the
    bank's first matmul since start zeroes the whole bank).
  - Self loops never enter the gather: an identity-matrix matmul adds the
    core's own Hs rows into each block's psum at segment 0.
  - GCN layers 2+3 feed only a global mean, so they collapse to weighted
    node sums (host-precomputed u2, S1, S2); p = sum_d u2[d]*relu(h1[d])
    is a PE reduction, AllReduced across cores; tiny MLP head replicated.
"""
import heapq
import numpy as np
import ml_dtypes

import concourse.bass as bass
import concourse.tile as tile
from concourse import bacc, mybir
from concourse.bass_utils import run_bass_kernel_spmd

N_CORES = 8
N = 100000
F = 128            # feature dim
BLK = 128          # dst-block size (psum partition dim)
NB = 98            # blocks per core
SHARD = NB * BLK   # 12544 rows per core (incl. pad rows)
ROWS = N_CORES * SHARD  # 100352 table rows
NSEG = 8           # segments per block
SEG = 256          # edges per segment (2 matmul groups)
GPB = NSEG * 2     # groups per block = 16
BPG = 14           # blocks per group (98 = 7*14)
CPP = 7            # block-groups
NCHUNK = NSEG * CPP
CH_FULL = BPG * SEG  # 3584 idxs
WIN = 32768
NQUEUES = 4
PAD_DST = 130.0

BF16 = ml_dtypes.bfloat16


def _win_base(k):
    return max(0, min(k * SHARD - 3584, ROWS - WIN))


# ----------------------------------------------------------------------------
# host preprocessing: renumber, normalize, sort, segment
# ----------------------------------------------------------------------------
def _preprocess(graph, edge_index, rates, params):
    src = np.asarray(edge_index[0], np.int64)
    dst = np.asarray(edge_index[1], np.int64)
    E = src.shape[0]

    # normalization scalars (f64, original ids; order-independent)
    deg = np.bincount(dst, minlength=N).astype(np.float64) + 1.0
    dinv = deg ** -0.5
    sq = deg ** 0.5
    u1 = dinv * (np.bincount(src, weights=dinv[dst], minlength=N) + dinv)
    y = u1 * dinv
    u2 = dinv * (np.bincount(src, weights=y[dst], minlength=N) + y)
    S1 = float(u1.sum())
    S2 = float(u2.sum())

    # LPT renumbering: balance per-block in-degree (target <= 2048)
    indeg = np.bincount(dst, minlength=N)
    nbins = N_CORES * NB
    order = np.argsort(-indeg, kind="stable")
    heap = [(0, 0, b) for b in range(nbins)]
    heapq.heapify(heap)
    binof = np.empty(N, np.int32)
    posof = np.empty(N, np.int32)
    for v in order:
        load, cnt, b = heapq.heappop(heap)
        binof[v] = b
        posof[v] = cnt
        cnt += 1
        load += int(indeg[v])
        if cnt < BLK:
            heapq.heappush(heap, (load, cnt, b))
    newid = binof.astype(np.int64) * BLK + posof
    # table row order is partition-major per core: row = c*SHARD + p*NB + b,
    # so the SBUF hs tile [128, NB, F] stores to DRAM as a single identity DMA
    c_ = newid // SHARD
    b_ = (newid % SHARD) // BLK
    p_ = newid % BLK
    trow = c_ * SHARD + p_ * NB + b_

    ns, nd = trow[src], newid[dst]
    cell = nd // BLK                      # global cell id = core*NB + block
    o = np.lexsort((ns, cell))
    cell_s, ns_s, off_s = cell[o], ns[o], (nd[o] % BLK).astype(np.float64)
    ccnt = np.bincount(cell_s, minlength=nbins)
    assert ccnt.max() <= NSEG * SEG, f"cell overflow {ccnt.max()}"
    cstart = np.zeros(nbins + 1, np.int64)
    cstart[1:] = np.cumsum(ccnt)

    # per-core edge tables in schedule order (block-group-major: ci = bg*8+k)
    idx16 = np.zeros((N_CORES, NCHUNK, 128, CH_FULL // 16), np.int16)
    dstid = np.full((N_CORES, NCHUNK, 128, CH_FULL // 128), PAD_DST, BF16)
    iv = np.zeros(CH_FULL, np.int64)
    ov = np.zeros(CH_FULL, np.float64)
    for c in range(N_CORES):
        for bg in range(CPP):
            blo, bhi = bg * BPG, (bg + 1) * BPG
            ncell = bhi - blo
            chn = ncell * SEG
            for k in range(NSEG):
                base = _win_base(k)
                iv[:chn] = 0
                ov[:chn] = PAD_DST
                for ci2 in range(ncell):
                    g_cell = c * NB + blo + ci2
                    s0, T = cstart[g_cell], ccnt[g_cell]
                    a, b2 = k * SEG, min((k + 1) * SEG, T)
                    if a >= b2:
                        continue
                    take = b2 - a
                    rel = ns_s[s0 + a : s0 + b2] - base
                    assert rel.min() >= 0 and rel.max() < WIN, (c, k, bg, ci2)
                    p0 = ci2 * SEG
                    iv[p0 : p0 + take] = rel
                    ov[p0 : p0 + take] = off_s[s0 + a : s0 + b2]
                ci = bg * NSEG + k
                e = np.arange(chn)
                tmp = np.zeros((16, CH_FULL // 16), np.int16)
                tmp[e % 16, e // 16] = iv[:chn]
                idx16[c, ci] = np.tile(tmp, (8, 1))
                dstid[c, ci][e % 128, e // 128] = ov[:chn].astype(BF16)

    # phase A inputs (new ordering, padded)
    X = np.asarray(graph, np.float32)
    inv = np.full(ROWS, -1, np.int64)
    inv[newid] = np.arange(N)
    xt = np.zeros((N_CORES, F, SHARD), np.float32)
    dinv_pm = np.zeros((N_CORES, BLK, NB), np.float32)
    u2_pm = np.zeros((N_CORES, BLK, NB), np.float32)
    sqdeg = np.zeros((N_CORES, 1, SHARD), np.float32)
    dv = np.zeros(ROWS)
    uv = np.zeros(ROWS)
    sv = np.zeros(ROWS)
    real = inv >= 0
    dv[real] = dinv[inv[real]]
    uv[real] = u2[inv[real]]
    sv[real] = sq[inv[real]]
    for c in range(N_CORES):
        rows = inv[c * SHARD : (c + 1) * SHARD]
        m = rows >= 0
        xt[c][:, m] = X[rows[m]].T
        dinv_pm[c] = dv[c * SHARD : (c + 1) * SHARD].reshape(NB, BLK).T
        u2_pm[c] = uv[c * SHARD : (c + 1) * SHARD].reshape(NB, BLK).T
        sqdeg[c, 0] = sv[c * SHARD : (c + 1) * SHARD]

    p = params
    col = lambda v: np.asarray(v, np.float32).reshape(-1, 1)
    iota = np.tile(np.arange(BLK, dtype=BF16)[None, :], (128, 1))
    ident = np.eye(128, dtype=BF16)
    common = dict(
        w1=np.asarray(p["conv1_W"], np.float32),
        b1row=np.asarray(p["conv1_b"], BF16).reshape(1, F),
        iota=iota,
        ident=ident,
        rates_col=col(rates),
        encw1=np.asarray(p["enc_W1"], np.float32),
        encb1=col(p["enc_b1"]),
        encw2=np.asarray(p["enc_W2"], np.float32),
        encb2=col(p["enc_b2"]),
        w2a=np.asarray(p["conv2_W"], np.float32)[:F],
        w2b=np.asarray(p["conv2_W"], np.float32)[F:],
        b2col=col(p["conv2_b"]),
        s1col=np.full((F, 1), S1, np.float32),
        s2col=np.full((F, 1), S2, np.float32),
        w3=np.asarray(p["conv3_W"], np.float32),
        b3col=col(p["conv3_b"]),
        hidw=np.asarray(p["hid_W"], np.float32),
        hidb=np.asarray(p["hid_b"], np.float32).reshape(2, F).T,
        hid2wa=np.asarray(p["hid2_W"], np.float32)[:F],
        hid2wb=np.asarray(p["hid2_W"], np.float32)[F:],
        hid2b=col(p["hid2_b"]),
        finw=np.asarray(p["fin_W"], np.float32),
        finb=col(p["fin_b"]),
    )
    in_maps = []
    for c in range(N_CORES):
        m = dict(common)
        m.update(
            xt=xt[c], sqdeg=sqdeg[c].astype(BF16), dinv=dinv_pm[c],
            u2c=u2_pm[c], idx16=idx16[c],
            dstid=np.ascontiguousarray(dstid[c].transpose(1, 0, 2)),
        )
        in_maps.append(m)
    return in_maps


# ----------------------------------------------------------------------------
# device program
# ----------------------------------------------------------------------------
def _build():
    f32, bf16, i16 = mybir.dt.float32, mybir.dt.bfloat16, mybir.dt.int16

    nc = bacc.Bacc("TRN2", target_bir_lowering=False, debug=False,
                   num_devices=N_CORES, num_swdge_queues=NQUEUES)
    I = lambda name, shape, dt=f32: nc.dram_tensor(name, shape, dt, kind="ExternalInput")
    xt_e = I("xt", [F, SHARD])
    w1_e = I("w1", [F, F]); b1_e = I("b1row", [1, F], bf16)
    sq_e = I("sqdeg", [1, SHARD], bf16)
    dinv_e = I("dinv", [BLK, NB]); u2_e = I("u2c", [BLK, NB])
    idx_e = I("idx16", [NCHUNK, 128, CH_FULL // 16], i16)
    dst_e = I("dstid", [128, NCHUNK, CH_FULL // 128], bf16)
    iota_e = I("iota", [128, BLK], bf16)
    ident_e = I("ident", [128, 128], bf16)
    rates_e = I("rates_col", [16, 1])
    encw1_e = I("encw1", [16, 8]); encb1_e = I("encb1", [8, 1])
    encw2_e = I("encw2", [8, F]); encb2_e = I("encb2", [F, 1])
    w2a_e = I("w2a", [F, F]); w2b_e = I("w2b", [F, F]); b2_e = I("b2col", [F, 1])
    s1_e = I("s1col", [F, 1]); s2_e = I("s2col", [F, 1])
    w3_e = I("w3", [F, F]); b3_e = I("b3col", [F, 1])
    hidw_e = I("hidw", [F, 2 * F]); hidb_e = I("hidb", [F, 2])
    hid2wa_e = I("hid2wa", [F, F]); hid2wb_e = I("hid2wb", [F, F])
    hid2b_e = I("hid2b", [F, 1])
    finw_e = I("finw", [F, 2]); finb_e = I("finb", [2, 1])
    out_e = nc.dram_tensor("out", [2, 1], f32, kind="ExternalOutput")

    warm_d = nc.dram_tensor("warm_d", [1, 4], f32)
    warm_s = nc.dram_tensor("warm_s", [1, 4], f32, addr_space="Shared")
    hs_shard = nc.dram_tensor("hs_shard", [128, NB * F], bf16)
    hs_full = nc.dram_tensor("hs_full", [ROWS, F], bf16, addr_space="Shared")
    p_dram = nc.dram_tensor("p_dram", [1, F], f32)
    p_shared = nc.dram_tensor("p_shared", [1, F], f32, addr_space="Shared")
    groups_all = list(range(N_CORES))

    with tile.TileContext(nc) as tc:
        with (
            tc.tile_pool(name="const", bufs=1) as cpool,
            tc.tile_pool(name="xt", bufs=3) as xtpool,
            tc.tile_pool(name="hself", bufs=1) as hpool,
            tc.tile_pool(name="work", bufs=8) as wpool,
            tc.tile_pool(name="gat", bufs=10) as gpool,
            tc.tile_pool(name="sstile", bufs=4) as spool,
            tc.tile_pool(name="ps", bufs=2, space="PSUM") as pspool,
            tc.tile_pool(name="cellps", bufs=1, space="PSUM") as cpspool,
            tc.tile_pool(name="psp", bufs=1, space="PSUM") as psppool,
        ):
            # ---- warm up the collective stream (overlaps phase A)
            nc.gpsimd.collective_compute(
                "AllReduce", mybir.AluOpType.add,
                replica_groups=[groups_all],
                ins=[warm_d[:]], outs=[warm_s[:]],
            )
            # ---- constants
            w1_sb = cpool.tile([F, F], f32); nc.sync.dma_start(w1_sb[:], w1_e[:])
            b1_sb = cpool.tile([1, F], bf16); nc.sync.dma_start(b1_sb[:], b1_e[:])
            sq_sb = cpool.tile([1, SHARD], bf16); nc.sync.dma_start(sq_sb[:], sq_e[:])
            dinv_sb = cpool.tile([BLK, NB], f32); nc.sync.dma_start(dinv_sb[:], dinv_e[:])
            u2_sb = cpool.tile([BLK, NB], f32); nc.sync.dma_start(u2_sb[:], u2_e[:])
            iota_sb = cpool.tile([128, BLK], bf16); nc.sync.dma_start(iota_sb[:], iota_e[:])
            ident_sb = cpool.tile([128, 128], bf16); nc.sync.dma_start(ident_sb[:], ident_e[:])
            # all chunks' dst-offset columns, preloaded in one DMA
            dsta_sb = cpool.tile([128, NCHUNK * (CH_FULL // 128)], bf16)
            nc.sync.dma_start(dsta_sb[:], dst_e[:])

            # ---- phase A: Hs = dinv * (X @ W1), bf16, kept in SBUF
            hs_self = hpool.tile([128, NB, F], bf16)
            for i in range(14):
                xts = xtpool.tile([F, 7 * BLK], f32, tag="xts")
                nc.sync.dma_start(xts[:], xt_e[:, i * 7 * BLK : (i + 1) * 7 * BLK])
                for j in range(7):
                    b = i * 7 + j
                    psA = pspool.tile([BLK, F], f32, tag="ps")
                    nc.tensor.matmul(psA[:], xts[:, j * BLK : (j + 1) * BLK],
                                     w1_sb[:], start=True, stop=True)
                    nc.scalar.activation(
                        hs_self[:, b, :], psA[:], mybir.ActivationFunctionType.Copy,
                        scale=dinv_sb[:, b : b + 1],
                    )
                if i % 2 == 1:
                    # store finished 14-block stretch while phase A continues
                    lo = (i - 1) * 7 * F
                    nc.sync.dma_start(hs_shard[:, lo : lo + 14 * F],
                                      hs_self[:, (i - 1) * 7 : (i + 1) * 7, :])

            # ---- AllGather the node table
            nc.gpsimd.collective_compute(
                "AllGather", mybir.AluOpType.bypass,
                replica_groups=[groups_all],
                ins=[hs_shard[:]], outs=[hs_full[:]],
            )

            # ---- message passing: 7 block-groups x 8 segment chunks
            # each block's 16 matmul groups accumulate purely in PSUM
            ps_p = psppool.tile([1, F], f32)
            for bg in range(CPP):
                blo = bg * BPG
                ncell = BPG
                chn = ncell * SEG
                psbs = {}
                for t in range((ncell + 3) // 4):
                    pst_new = cpspool.tile([BLK, 4 * F], f32, tag=f"cps{t}")
                    for q in range(min(4, ncell - t * 4)):
                        psbs[blo + t * 4 + q] = pst_new[:, q * F : (q + 1) * F]
                for k in range(NSEG):
                    ci = bg * NSEG + k
                    base = _win_base(k)
                    idxt = wpool.tile([128, CH_FULL // 16], i16, tag="idxt")
                    nc.sync.dma_start(idxt[:, : chn // 16], idx_e[ci, :, : chn // 16])
                    G = gpool.tile([128, CH_FULL // 128, F], bf16, tag="G")
                    nc.gpsimd.dma_gather(
                        out_ap=G[:, : chn // 128, :],
                        in_ap=hs_full[base : base + WIN, :],
                        idxs_ap=idxt[:, : chn // 16],
                        num_idxs=chn, num_idxs_reg=chn, elem_size=F,
                        single_packet=False, queue_num=(ci % NQUEUES),
                    )
                    Sts = []
                    for sg in range((chn // 128 + 3) // 4):
                        S4 = spool.tile([128, 4, BLK], bf16, tag=f"S{sg % 2}")
                        iap = iota_sb[:]
                        in0 = bass.AP(iap.tensor, iap.offset,
                                      [iap.ap[0], [0, 4], iap.ap[1]])
                        c0 = ci * (CH_FULL // 128) + sg * 4
                        dap = dsta_sb[:, c0 : c0 + 4]
                        in1 = bass.AP(dap.tensor, dap.offset,
                                      [dap.ap[0], dap.ap[1], [0, 128]])
                        nc.vector.tensor_tensor(S4[:], in0, in1,
                                                mybir.AluOpType.is_equal)
                        Sts.append(S4)
                    for ci2 in range(ncell):
                        b = blo + ci2
                        psb = psbs[b]
                        if k == 0:
                            # bias (sqrt(deg) (x) b1) + self-loop rows.
                            # start=True zeroes the WHOLE psum bank, so only
                            # the first slice of each 4-block bank sets it.
                            nc.tensor.matmul(
                                psb[:], sq_sb[:, b * BLK : (b + 1) * BLK],
                                b1_sb[:], start=(ci2 % 4 == 0), stop=False,
                                skip_group_check=True,
                            )
                            nc.tensor.matmul(
                                psb[:], ident_sb[:], hs_self[:, b, :],
                                start=False, stop=False, skip_group_check=True,
                            )
                        for g2 in range(2):
                            g = ci2 * 2 + g2
                            nc.tensor.matmul(
                                psb[:], Sts[g // 4][:, g % 4, :], G[:, g, :],
                                start=False, stop=(k == NSEG - 1 and g2 == 1),
                                skip_group_check=True,
                            )
                    if k == NSEG - 1:
                        for ci2 in range(ncell):
                            b = blo + ci2
                            h1b = spool.tile([BLK, F], f32, tag="h1b")
                            nc.scalar.activation(
                                h1b[:], psbs[b][:],
                                mybir.ActivationFunctionType.Relu,
                                scale=dinv_sb[:, b : b + 1],
                            )
                            nc.tensor.matmul(
                                ps_p[:], u2_sb[:, b : b + 1], h1b[:],
                                start=(b == 0), stop=(b == NB - 1),
                                skip_group_check=True,
                            )

            # ---- p AllReduce
            p_sb = cpool.tile([1, F], f32)
            nc.vector.tensor_copy(p_sb[:], ps_p[:])
            nc.sync.dma_start(p_dram[:], p_sb[:])
            nc.gpsimd.collective_compute(
                "AllReduce", mybir.AluOpType.add,
                replica_groups=[groups_all],
                ins=[p_dram[:]], outs=[p_shared[:]],
            )
            p_row = cpool.tile([1, F], f32)
            nc.sync.dma_start(p_row[:], p_shared[:])
            id1 = cpool.tile([1, 1], f32)
            nc.vector.memset(id1[:], 1.0)
            psT = pspool.tile([F, 1], f32, tag="ps")
            nc.tensor.transpose(psT[:], p_row[:], id1[:])
            p_col = cpool.tile([F, 1], f32)
            nc.vector.tensor_copy(p_col[:], psT[:])

            # ---- replicated tail MLP
            tl = cpool
            def ld(e, shape, dt=f32):
                t = tl.tile(shape, dt, tag=f"c_{e.name}")
                nc.sync.dma_start(t[:], e[:])
                return t
            rates_sb = ld(rates_e, [16, 1]); encw1_sb = ld(encw1_e, [16, 8])
            encb1_sb = ld(encb1_e, [8, 1]); encw2_sb = ld(encw2_e, [8, F])
            encb2_sb = ld(encb2_e, [F, 1])
            w2a_sb = ld(w2a_e, [F, F]); w2b_sb = ld(w2b_e, [F, F])
            b2_sb = ld(b2_e, [F, 1]); s1_sb = ld(s1_e, [F, 1]); s2_sb = ld(s2_e, [F, 1])
            w3_sb = ld(w3_e, [F, F]); b3_sb = ld(b3_e, [F, 1])
            hidw_sb = ld(hidw_e, [F, 2 * F]); hidb_sb = ld(hidb_e, [F, 2])
            hid2wa_sb = ld(hid2wa_e, [F, F]); hid2wb_sb = ld(hid2wb_e, [F, F])
            hid2b_sb = ld(hid2b_e, [F, 1])
            finw_sb = ld(finw_e, [F, 2]); finb_sb = ld(finb_e, [2, 1])

            pst = pspool.tile([F, 2], f32, tag="ps")
            nc.tensor.matmul(pst[:8, 0:1], encw1_sb[:], rates_sb[:], start=True, stop=True)
            r1 = tl.tile([8, 1], f32)
            nc.scalar.activation(r1[:], pst[:8, 0:1],
                                 mybir.ActivationFunctionType.Relu, bias=encb1_sb[:])
            nc.tensor.matmul(pst[:, 1:2], encw2_sb[:], r1[:], start=True, stop=True)
            r2 = tl.tile([F, 1], f32)
            nc.vector.tensor_add(r2[:], pst[:, 1:2], encb2_sb[:])
            mr = tl.tile([F, 1], f32)
            nc.vector.tensor_mul(mr[:], r2[:], s2_sb[:])
            pst2 = pspool.tile([F, 1], f32, tag="ps")
            nc.tensor.matmul(pst2[:], w2a_sb[:], p_col[:], start=True, stop=False)
            nc.tensor.matmul(pst2[:], w2b_sb[:], mr[:], start=False, stop=True)
            sb2 = tl.tile([F, 1], f32)
            nc.vector.tensor_mul(sb2[:], b2_sb[:], s1_sb[:])
            qv = tl.tile([F, 1], f32)
            nc.vector.tensor_add(qv[:], pst2[:], sb2[:])
            nc.vector.tensor_scalar_mul(qv[:], qv[:], 1.0 / N)
            pst3 = pspool.tile([F, 1], f32, tag="ps")
            nc.tensor.matmul(pst3[:], w3_sb[:], qv[:], start=True, stop=True)
            m3 = tl.tile([F, 1], f32)
            nc.vector.tensor_add(m3[:], pst3[:], b3_sb[:])
            g1a = tl.tile([F, 1], f32); g1b = tl.tile([F, 1], f32)
            nc.tensor.matmul(pst[:, 0:1], hidw_sb[:, :F], m3[:], start=True, stop=True)
            nc.scalar.activation(g1a[:], pst[:, 0:1],
                                 mybir.ActivationFunctionType.Relu, bias=hidb_sb[:, 0:1])
            nc.tensor.matmul(pst[:, 1:2], hidw_sb[:, F:], m3[:], start=True, stop=True)
            nc.scalar.activation(g1b[:], pst[:, 1:2],
                                 mybir.ActivationFunctionType.Relu, bias=hidb_sb[:, 1:2])
            pst4 = pspool.tile([F, 1], f32, tag="ps")
            nc.tensor.matmul(pst4[:], hid2wa_sb[:], g1a[:], start=True, stop=False)
            nc.tensor.matmul(pst4[:], hid2wb_sb[:], g1b[:], start=False, stop=True)
            g2 = tl.tile([F, 1], f32)
            nc.scalar.activation(g2[:], pst4[:],
                                 mybir.ActivationFunctionType.Relu, bias=hid2b_sb[:])
            pst5 = pspool.tile([2, 1], f32, tag="ps")
            nc.tensor.matmul(pst5[:], finw_sb[:], g2[:], start=True, stop=True)
            outv = tl.tile([2, 1], f32)
            nc.vector.tensor_add(outv[:], pst5[:], finb_sb[:])
            nc.sync.dma_start(out_e[:], outv[:])

    nc.compile()
    return nc


_CACHE = {}
LAST_RESULTS = None


def kernel(**inputs):
    graph = np.asarray(inputs["graph"], np.float32)
    edge_index = np.asarray(inputs["edge_index"], np.int64)
    rates = np.asarray(inputs["rates"], np.float32)
    params = {k: np.asarray(v) for k, v in inputs.items()
              if k not in ("graph", "edge_index", "rates")}
    in_maps = _preprocess(graph, edge_index, rates, params)
    if "nc" not in _CACHE:
        _CACHE["nc"] = _build()
    nc = _CACHE["nc"]
    import os
    trace = bool(int(os.environ.get("GCN_TRACE", "0")))
    res = run_bass_kernel_spmd(nc, in_maps, list(range(N_CORES)), trace=trace)
    global LAST_RESULTS
    LAST_RESULTS = res
    out = np.asarray(res.results[0]["out"], np.float32).reshape(1, 2)
    return out
